# revision 32
# baseline (speedup 1.0000x reference)
"""LEM cell (ODE2) Bass kernel for Trainium2, 8-core data-parallel, fp8 GEMMs.

Math (per batch row b):
  ti = x @ W_ih.T + b_ih                  # [B, 4H]
  th = y @ W_hh.T + b_hh                  # [B, 3H]
  tdt = dt @ W_dt.T + b_dt                # [B, 2]
  ms_dt_bar = sig(tdt[:,0]) * sig(ti[:, :H]   + th[:, :H])
  ms_dt     = sig(tdt[:,1]) * sig(ti[:, H:2H] + th[:, H:2H])
  z_new = (1-ms_dt) * z + ms_dt * tanh(ti[:, 3H:] + th[:, 2H:3H])
  y_new = (1-ms_dt_bar) * y + ms_dt_bar * tanh(z_new @ W_z.T + b_z + ti[:, 2H:3H])
  returns (y_new, z_new)

Strategy: shard batch across 8 cores (2048 rows each). On-chip everything is
feature-major ([feature_tile=128 partitions, batch columns free]); the host
pre-transposes activations and pre-packs weights into stationary blocks.
All GEMMs run in fp8 e4m3 with MatmulPerfMode.DoubleRow (two 128-row
contraction blocks per matmul, 2x bf16 PE rate). Activations are scaled
x16, weights x1024 before the e4m3 cast; the 2^-14 dequant rides in the
PSUM-consuming activation's `scale` operand. The i+h sums and the
i_z + z_new@W_z.T sum accumulate into the same PSUM bank. z / y for the
element-wise paths travel separately as fp32 / bf16, outputs fp32.

DMA descriptor generation costs ~620ns of sequencer time per dma_start,
so transfers are merged aggressively: x/y are host-packed pair-major
([pair, 128, 2, Bs]) so one DMA fills a DoubleRow rhs tile, weights load
as single full stationary tiles, z / y-elementwise load per-jt rows, and
outputs accumulate in half-panel SBUF tiles before one store each.
Measured fp8 error vs the fp32 reference: rel ~1.4e-2 (absmax metric).
"""

import sys

_REPO = "/opt/trn_rl_repo"
if _REPO not in sys.path:
    sys.path.insert(0, _REPO)

from contextlib import ExitStack

import numpy as np
import ml_dtypes

import concourse.bacc as bacc
import concourse.bass as bass
import concourse.tile as tile
from concourse import mybir
from concourse.bass_utils import run_bass_kernel_spmd

P = 128
F32 = mybir.dt.float32
F8 = mybir.dt.float8e4
BF16 = mybir.dt.bfloat16
AF = mybir.ActivationFunctionType
DR = mybir.MatmulPerfMode.DoubleRow
NP_F8 = ml_dtypes.float8_e4m3
NP_BF16 = ml_dtypes.bfloat16

N_CORES = 8
NINP = 1024
NHID = 1024
BATCH = 16384

SA = 16.0     # activation quant scale (|x|max ~5.6 -> 90 < 240)
SW = 1024.0   # weight quant scale (|w|max 1/32 -> 32 < 240)
DQ = 1.0 / (SA * SW)  # 2^-14, folded into the PSUM-reading activation

LAST_RESULTS = None  # BassKernelResults of the most recent kernel() call


def build_nc(
    K,            # input feature dim (x)
    H,            # hidden dim (y/z)
    B_shard,      # batch rows per core
    panel,        # batch columns kept resident per pass (== B_shard here)
    chunk,        # matmul moving-dim size (512 = one fp32 PSUM bank)
    wdt00, wdt10,  # W_dt scalars (baked immediates; b_dt rides in biasP)
    w_bufs=23,
    ps_bufs=8,
):
    NJT = H // P          # output feature tiles (per H-sized group)
    NKP = K // (2 * P)    # x-side contraction pair-tiles (DoubleRow)
    NHP = H // (2 * P)    # y/z-side contraction pair-tiles
    NWT = (K + H) // P    # weight tile columns (packed pairs layout)
    npan = B_shard // panel
    nch = panel // chunk
    half = panel // 2     # output tiles cover half panels

    nc = bacc.Bacc(trn_type="TRN2", target_bir_lowering=False)

    # pair-major activations: [pair, 128, 2, B] so one DMA fills a rhs tile
    x8P = nc.declare_dram_parameter("x8P", [NKP, P, 2, B_shard], F8, isOutput=False)
    y8P = nc.declare_dram_parameter("y8P", [NHP, P, 2, B_shard], F8, isOutput=False)
    ybT = nc.declare_dram_parameter("ybT", [H, B_shard], BF16, isOutput=False)
    zT = nc.declare_dram_parameter("zT", [H, B_shard], F32, isOutput=False)
    dtr = nc.declare_dram_parameter("dtr", [1, B_shard], F32, isOutput=False)
    # packed stationary blocks: [jt, kin, (kt_a*P+j | kt_b*P+j)] fp8
    Wd2 = nc.declare_dram_parameter("Wd2", [NJT, P, K + H], F8, isOutput=False)
    Wy = nc.declare_dram_parameter("Wy", [NJT, P, K + H], F8, isOutput=False)
    Wd1 = nc.declare_dram_parameter("Wd1", [NJT, P, K + H], F8, isOutput=False)
    Wg3 = nc.declare_dram_parameter("Wg3", [NJT, P, K + H], F8, isOutput=False)
    # last two columns: row 0 holds b_dt[0], b_dt[1]
    biasP = nc.declare_dram_parameter("biasP", [P, 4 * NJT + 2], F32, isOutput=False)

    y_newT = nc.declare_dram_parameter("y_newT", [H, B_shard], F32, isOutput=True)
    z_newT = nc.declare_dram_parameter("z_newT", [H, B_shard], F32, isOutput=True)

    assert npan == 1, "single-panel schedule (whole shard resident)"

    with tile.TileContext(nc) as tc, ExitStack() as ctx:
        cpool = ctx.enter_context(tc.tile_pool(name="cpool", bufs=1))
        xpool = ctx.enter_context(tc.tile_pool(name="xpool", bufs=NKP))
        ypool = ctx.enter_context(tc.tile_pool(name="ypool", bufs=NHP))
        ybpool = ctx.enter_context(tc.tile_pool(name="ybpool", bufs=2))
        zpool = ctx.enter_context(tc.tile_pool(name="zpool", bufs=2))
        znpool = ctx.enter_context(tc.tile_pool(name="znpool", bufs=NHP))
        wpool = ctx.enter_context(tc.tile_pool(name="wpool", bufs=w_bufs))
        apool = ctx.enter_context(tc.tile_pool(name="apool", bufs=3))
        dpool = ctx.enter_context(tc.tile_pool(name="dpool", bufs=3))
        opool = ctx.enter_context(tc.tile_pool(name="opool", bufs=2))
        bcpool = ctx.enter_context(tc.tile_pool(name="bcpool", bufs=1))
        rpool = ctx.enter_context(tc.tile_pool(name="rpool", bufs=2))
        pspool = ctx.enter_context(tc.tile_pool(name="pspool", bufs=ps_bufs, space="PSUM"))

        bias_sb = cpool.tile([P, 4 * NJT + 2], F32, name="bias_sb")
        nc.sync.dma_start(bias_sb[:], biasP[:, :])

        def bias_ap(g, jt):
            i = g * NJT + jt
            return bias_sb[:, i : i + 1]

        b0 = 0

        def col(c, n=1):
            return slice(b0 + c * chunk, b0 + (c + n) * chunk)

        dt_sb = rpool.tile([1, panel], F32, name="dt_sb", tag="dtr", bufs=1)
        nc.sync.dma_start(dt_sb[:], dtr[0:1, b0 : b0 + panel])

        # per-batch dt gates first: tiny ACT ops must precede the input
        # DMA flood in the ACT FIFO, else bc gates arrive ~40us late
        sg1 = rpool.tile([1, panel], F32, name="sg1", tag="sg")
        nc.scalar.activation(
            sg1[:], dt_sb[:], AF.Sigmoid,
            bias=bias_sb[0:1, 4 * NJT : 4 * NJT + 1], scale=wdt00,
        )
        sg2 = rpool.tile([1, panel], F32, name="sg2", tag="sg")
        nc.scalar.activation(
            sg2[:], dt_sb[:], AF.Sigmoid,
            bias=bias_sb[0:1, 4 * NJT + 1 : 4 * NJT + 2], scale=wdt10,
        )

        x_t = [None] * NKP
        y_t = [None] * NHP
        z_t = [None] * NJT
        yb_t = [None] * NJT

        def load_z(jt, eng):
            z_sb = zpool.tile([P, panel], F32, name="z_sb", tag="z")
            eng.dma_start(z_sb[:], zT[jt * P : (jt + 1) * P, b0 : b0 + panel])
            z_t[jt] = z_sb

        def load_yb(jt, eng):
            yb_sb = ybpool.tile([P, panel], BF16, name="yb_sb", tag="yb")
            eng.dma_start(yb_sb[:], ybT[jt * P : (jt + 1) * P, b0 : b0 + panel])
            yb_t[jt] = yb_sb

        def load_w(Wsrc, jt, name, eng):
            w_sb = wpool.tile([P, NWT, P], F8, name=name, tag="w")
            eng.dma_start(w_sb[:], Wsrc[jt][:, :])
            return w_sb

        # ---- cold-start staging ----
        # x pair-tiles span the whole panel; q0's first chunk arrives first
        # (gpsimd), everything else streams on the otherwise-idle DVE queue
        for q in range(NKP):
            x_t[q] = xpool.tile([P, 2, panel], F8, name="xt", tag="xt")
        # x pair q0 leads on gpsimd; q1-3 spread over sync/scalar so their
        # transfers run in parallel with q0's (per-queue DMA is serial)
        nc.gpsimd.dma_start(x_t[0][:, :, 0:chunk], x8P[0][:, :, 0:chunk])
        nc.gpsimd.dma_start(x_t[0][:, :, chunk:panel], x8P[0][:, :, chunk:panel])

        # spread the cold-start bytes across all three DMA queues in
        # consumption order (per-queue transfers drain serially ~200GB/s)
        for q in range(NHP):
            y_t[q] = ypool.tile([P, 2, panel], F8, name="yt", tag="yt")
        wB = [[None, None] for _ in range(NJT)]   # per jt: [Wd2, Wy]
        wC = [[None, None] for _ in range(NJT)]   # per jt: [Wd1, Wg3]
        wB[0][0] = load_w(Wd2, 0, "wd2_sb", nc.sync)
        wB[0][1] = load_w(Wy, 0, "wy_sb", nc.scalar)
        for q in range(1, NKP):
            nc.scalar.dma_start(x_t[q][:], x8P[q][:, :, :])
        for q in range(NHP):
            nc.sync.dma_start(y_t[q][:], y8P[q][:, :, :])
        # bc broadcasts ahead of the bulky z loads (bc2 is needed by the
        # first DVE op at ~24us, z0 not till ~26us)
        bc1 = bcpool.tile([P, panel], F32, name="bc1", tag="bc1")
        nc.gpsimd.partition_broadcast(bc1[:], sg1[0:1, :])
        bc2 = bcpool.tile([P, panel], F32, name="bc2", tag="bc2")
        nc.gpsimd.partition_broadcast(bc2[:], sg2[0:1, :])
        load_z(0, nc.gpsimd)
        load_z(1, nc.gpsimd)
        for jt in range(1, NJT):
            wB[jt][0] = load_w(Wd2, jt, "wd2_sb", nc.sync)
            wB[jt][1] = load_w(Wy, jt, "wy_sb", nc.scalar)

        kb = K // P

        def x_of(q, c):
            return x_t[q][:, :, c * chunk : (c + 1) * chunk]

        def y_of(q, c):
            return y_t[q][:, :, c * chunk : (c + 1) * chunk]

        def zn_of(q, c):
            return zn_t[q][:, :, c * chunk : (c + 1) * chunk]

        def accum_blocks(pss, w_sb, rhs_of, side, n_q, start=False, stop=False):
            """Block-outer fp8 DoubleRow accumulation: for each stationary
            pair-block q, matmul all chunks' psums before moving on, so the
            PE array reuses the loaded weights across `nch` moving passes."""
            base = 0 if side == 0 else kb
            for q in range(n_q):
                lhsT = w_sb[:, base + 2 * q : base + 2 * q + 2, :]
                for c in range(len(pss)):
                    nc.tensor.matmul(
                        pss[c][:],
                        lhsT=lhsT,
                        rhs=rhs_of(q, c),
                        start=start and (q == 0),
                        stop=stop and (q == n_q - 1),
                        perf_mode=DR,
                    )

        # ---- phase B: d2 + y gates -> z_new ----
        # zn pair-tiles [P, 2, panel]: pair q holds jt=2q / 2q+1 rows
        zn_t = [
            znpool.tile([P, 2, panel], F8, name="znr", tag="zn")
            for _ in range(NHP)
        ]
        for jt in range(NJT):
            wd2_sb, wy_sb = wB[jt]
            if jt + 2 < NJT:
                load_z(jt + 2, nc.gpsimd)
            elif jt + 2 - NJT < NJT:
                load_yb(jt + 2 - NJT, nc.gpsimd)  # yb0/yb1 prefetch late in B
            # stage phase-C stationary tiles through B's back half
            if 2 <= jt < 2 + NJT // 2:
                j2 = 2 * (jt - 2)
                wC[j2][0] = load_w(Wd1, j2, "wd1_sb", nc.sync)
                wC[j2][1] = load_w(Wg3, j2, "wg3_sb", nc.scalar)
                wC[j2 + 1][0] = load_w(Wd1, j2 + 1, "wd1_sb", nc.sync)
                wC[j2 + 1][1] = load_w(Wg3, j2 + 1, "wg3_sb", nc.scalar)
            zo = opool.tile([P, panel], F32, name="zo", tag="zo")
            ps1s = [pspool.tile([P, chunk], F32, name="ps1", tag="ps") for _ in range(nch)]
            ps2s = [pspool.tile([P, chunk], F32, name="ps2", tag="ps") for _ in range(nch)]
            # x-sides first (block-outer): PE chews these while y streams
            accum_blocks(ps1s, wd2_sb, x_of, 0, NKP, start=True)
            accum_blocks(ps2s, wy_sb, x_of, 0, NKP, start=True)
            accum_blocks(ps1s, wd2_sb, y_of, 1, NHP, stop=True)
            accum_blocks(ps2s, wy_sb, y_of, 1, NHP, stop=True)
            for c in range(nch):
                cs = slice(c * chunk, (c + 1) * chunk)
                s2 = apool.tile([P, chunk], F32, name="s2", tag="act")
                nc.scalar.activation(s2[:], ps1s[c][:], AF.Sigmoid, bias=bias_ap(0, jt), scale=DQ)
                tz = apool.tile([P, chunk], F32, name="tz", tag="act")
                nc.scalar.activation(tz[:], ps2s[c][:], AF.Tanh, bias=bias_ap(1, jt), scale=DQ)

                # z_new = z + ms2*(tanh - z); 4 DVE ops
                ms2 = dpool.tile([P, chunk], F32, name="ms2", tag="dve")
                nc.vector.tensor_mul(ms2[:], s2[:], bc2[:, cs])
                dlt = dpool.tile([P, chunk], F32, name="dlt", tag="dve")
                nc.vector.tensor_sub(dlt[:], tz[:], z_t[jt][:, cs])
                prd = dpool.tile([P, chunk], F32, name="prd", tag="dve")
                nc.vector.tensor_mul(prd[:], ms2[:], dlt[:])
                nc.vector.tensor_add(zo[:, cs], prd[:], z_t[jt][:, cs])
                # quantizing cast (x16 -> e4m3) into the resident
                # DoubleRow pair-tile for GEMM3
                nc.scalar.mul(zn_t[jt // 2][:, jt % 2, cs], zo[:, cs], SA)
                if c % 2 == 1:
                    nc.sync.dma_start(
                        z_newT[jt * P : (jt + 1) * P, col(c - 1, 2)],
                        zo[:, (c - 1) * chunk : (c + 1) * chunk],
                    )

        # ---- phase C: d1 gate + (i_z + z_new @ W_z.T) -> y_new ----
        for jt in range(NJT):
            if wC[jt][0] is None:
                wC[jt][0] = load_w(Wd1, jt, "wd1_sb", nc.sync)
                wC[jt][1] = load_w(Wg3, jt, "wg3_sb", nc.scalar)
            wd1_sb, wg3_sb = wC[jt]
            if jt + 2 < NJT:
                load_yb(jt + 2, nc.gpsimd)
            yo = opool.tile([P, panel], F32, name="yo", tag="yo")

            def c_tail(c, ps3, ps4):
                cs = slice(c * chunk, (c + 1) * chunk)
                s1 = apool.tile([P, chunk], F32, name="s1", tag="act")
                nc.scalar.activation(s1[:], ps3[:], AF.Sigmoid, bias=bias_ap(2, jt), scale=DQ)
                u = apool.tile([P, chunk], F32, name="u", tag="act")
                nc.scalar.activation(u[:], ps4[:], AF.Tanh, bias=bias_ap(3, jt), scale=DQ)

                # y_new = y + ms1*(u - y); 4 DVE ops
                ms1 = dpool.tile([P, chunk], F32, name="ms1", tag="dve")
                nc.vector.tensor_mul(ms1[:], s1[:], bc1[:, cs])
                dly = dpool.tile([P, chunk], F32, name="dly", tag="dve")
                nc.vector.tensor_sub(dly[:], u[:], yb_t[jt][:, cs])
                mdy = dpool.tile([P, chunk], F32, name="mdy", tag="dve")
                nc.vector.tensor_mul(mdy[:], ms1[:], dly[:])
                nc.vector.tensor_add(yo[:, cs], mdy[:], yb_t[jt][:, cs])
                if c % 2 == 1:
                    nc.scalar.dma_start(
                        y_newT[jt * P : (jt + 1) * P, col(c - 1, 2)],
                        yo[:, (c - 1) * chunk : (c + 1) * chunk],
                    )

            if jt < NJT - 1:
                ps3s = [pspool.tile([P, chunk], F32, name="ps3", tag="ps") for _ in range(nch)]
                ps4s = [pspool.tile([P, chunk], F32, name="ps4", tag="ps") for _ in range(nch)]
                accum_blocks(ps3s, wd1_sb, x_of, 0, NKP, start=True)
                accum_blocks(ps4s, wg3_sb, x_of, 0, NKP, start=True)
                accum_blocks(ps3s, wd1_sb, y_of, 1, NHP, stop=True)
                accum_blocks(ps4s, wg3_sb, zn_of, 1, NHP, stop=True)
                for c in range(nch):
                    c_tail(c, ps3s[c], ps4s[c])
            else:
                # last jt: chunk-interleaved with per-chunk stores so the
                # ACT/DVE/store tail overlaps the remaining matmuls
                for c in range(nch):
                    cs = slice(c * chunk, (c + 1) * chunk)
                    ps3 = pspool.tile([P, chunk], F32, name="ps3", tag="ps")
                    accum_blocks([ps3], wd1_sb, lambda q, _c, c=c: x_of(q, c), 0, NKP, start=True)
                    accum_blocks([ps3], wd1_sb, lambda q, _c, c=c: y_of(q, c), 1, NHP, stop=True)
                    ps4 = pspool.tile([P, chunk], F32, name="ps4", tag="ps")
                    accum_blocks([ps4], wg3_sb, lambda q, _c, c=c: x_of(q, c), 0, NKP, start=True)
                    accum_blocks([ps4], wg3_sb, lambda q, _c, c=c: zn_of(q, c), 1, NHP, stop=True)
                    s1 = apool.tile([P, chunk], F32, name="s1", tag="act")
                    nc.scalar.activation(s1[:], ps3[:], AF.Sigmoid, bias=bias_ap(2, jt), scale=DQ)
                    u = apool.tile([P, chunk], F32, name="u", tag="act")
                    nc.scalar.activation(u[:], ps4[:], AF.Tanh, bias=bias_ap(3, jt), scale=DQ)
                    # alternate chunk chains across DVE and GpSimd so the
                    # trailing element-wise work drains on two engines
                    ve = nc.vector if c % 2 else nc.gpsimd
                    ms1 = dpool.tile([P, chunk], F32, name="ms1", tag="dve")
                    ve.tensor_mul(ms1[:], s1[:], bc1[:, cs])
                    if c < nch - 1:
                        dly = dpool.tile([P, chunk], F32, name="dly", tag="dve")
                        ve.tensor_sub(dly[:], u[:], yb_t[jt][:, cs])
                        mdy = dpool.tile([P, chunk], F32, name="mdy", tag="dve")
                        ve.tensor_mul(mdy[:], ms1[:], dly[:])
                        ve.tensor_add(yo[:, cs], mdy[:], yb_t[jt][:, cs])
                        nc.scalar.dma_start(
                            y_newT[jt * P : (jt + 1) * P, col(c)], yo[:, cs]
                        )
                    else:
                        # final chunk: (1-ms1)*y precomputed before the tanh
                        # lands, then a fused finish with one half on each
                        # engine so the last bytes store as early as possible
                        my = dpool.tile([P, chunk], F32, name="my", tag="dve")
                        nc.vector.tensor_mul(my[:], ms1[:], yb_t[jt][:, cs])
                        wyp = dpool.tile([P, chunk], F32, name="wyp", tag="dve")
                        nc.vector.tensor_sub(wyp[:], yb_t[jt][:, cs], my[:])
                        hf = chunk // 2
                        for h in range(2):
                            he = nc.gpsimd if h == 0 else nc.vector
                            so = nc.sync if h == 0 else nc.scalar
                            hs = slice(c * chunk + h * hf, c * chunk + (h + 1) * hf)
                            ho = slice(h * hf, (h + 1) * hf)
                            mu = dpool.tile([P, hf], F32, name="mu", tag="dvef", bufs=2)
                            he.tensor_mul(mu[:], ms1[:, ho], u[:, ho])
                            he.tensor_add(yo[:, hs], wyp[:, ho], mu[:])
                            so.dma_start(
                                y_newT[jt * P : (jt + 1) * P,
                                       b0 + c * chunk + h * hf : b0 + c * chunk + (h + 1) * hf],
                                yo[:, hs],
                            )

    nc.compile()
    return nc


def _q8(a, s):
    """Scale and round-to-nearest cast to e4m3."""
    return (np.asarray(a, np.float32) * s).astype(NP_F8)


def _pack_pair(Wa, Wb):
    """[jt, kin, kt*P+j] stationary-block packing of two row-major [out, in]
    weight matrices (lhsT blocks: lhsT[kin, j] = W[jt*P+j, kt*P+kin]).
    Consecutive kt pairs are exactly the DoubleRow [kin, 2, j] layout."""
    def pack(W):
        O, I = W.shape
        njt, nkt = O // P, I // P
        return (
            W.reshape(njt, P, nkt, P).transpose(0, 3, 2, 1).reshape(njt, P, I)
        )
    A = pack(Wa)
    B = pack(Wb)
    return np.ascontiguousarray(np.concatenate([A, B], axis=2))


def _pair_major(a8T):
    """[K, B] fp8 -> [K/256, 128, 2, B] DoubleRow pair-major layout."""
    K, B = a8T.shape
    return np.ascontiguousarray(
        a8T.reshape(K // 256, 2, P, B).transpose(0, 2, 1, 3)
    )


def pack_host_inputs(x, y, z, dt, W_ih, b_ih, W_hh, b_hh, W_z, b_z, b_dt, n_cores):
    """Shard batch across cores; pre-transpose + fp8-quantize activations;
    pack + fp8-quantize weights."""
    B, K = x.shape
    H = y.shape[1]
    NJT = H // P
    Bs = B // n_cores

    x8T = np.ascontiguousarray(_q8(x, SA).T)
    y8T = np.ascontiguousarray(_q8(y, SA).T)
    ybT = np.ascontiguousarray(np.asarray(y, np.float32).T.astype(NP_BF16))
    zT = np.ascontiguousarray(np.asarray(z, np.float32).T)
    dtrow = np.ascontiguousarray(np.asarray(dt, np.float32).reshape(1, B))

    Wd2 = _pack_pair(_q8(W_ih[H : 2 * H], SW), _q8(W_hh[H : 2 * H], SW))
    Wy = _pack_pair(_q8(W_ih[3 * H : 4 * H], SW), _q8(W_hh[2 * H : 3 * H], SW))
    Wd1 = _pack_pair(_q8(W_ih[0:H], SW), _q8(W_hh[0:H], SW))
    Wg3 = _pack_pair(_q8(W_ih[2 * H : 3 * H], SW), _q8(W_z, SW))

    def bias_cols(bvec):
        return bvec.reshape(NJT, P).T  # [P, NJT]

    bdt_cols = np.zeros((P, 2), np.float32)
    bdt_cols[0, 0] = b_dt[0]
    bdt_cols[0, 1] = b_dt[1]
    biasP = np.ascontiguousarray(
        np.concatenate(
            [
                bias_cols(b_ih[H : 2 * H] + b_hh[H : 2 * H]),
                bias_cols(b_ih[3 * H : 4 * H] + b_hh[2 * H : 3 * H]),
                bias_cols(b_ih[0:H] + b_hh[0:H]),
                bias_cols(b_ih[2 * H : 3 * H] + b_z),
                bdt_cols,
            ],
            axis=1,
        ),
        dtype=np.float32,
    )

    in_maps = []
    for c in range(n_cores):
        cs = slice(c * Bs, (c + 1) * Bs)
        in_maps.append(
            {
                "x8P": _pair_major(x8T[:, cs]),
                "y8P": _pair_major(y8T[:, cs]),
                "ybT": np.ascontiguousarray(ybT[:, cs]),
                "zT": np.ascontiguousarray(zT[:, cs]),
                "dtr": np.ascontiguousarray(dtrow[:, cs]),
                "Wd2": Wd2,
                "Wy": Wy,
                "Wd1": Wd1,
                "Wg3": Wg3,
                "biasP": biasP,
            }
        )
    return in_maps


def kernel(x, y, z, dt, W_ih, b_ih, W_hh, b_hh, W_z, b_z, W_dt, b_dt):
    x = np.asarray(x, np.float32)
    y = np.asarray(y, np.float32)
    z = np.asarray(z, np.float32)
    dt = np.asarray(dt, np.float32)
    W_ih = np.asarray(W_ih, np.float32)
    b_ih = np.asarray(b_ih, np.float32)
    W_hh = np.asarray(W_hh, np.float32)
    b_hh = np.asarray(b_hh, np.float32)
    W_z = np.asarray(W_z, np.float32)
    b_z = np.asarray(b_z, np.float32)
    W_dt = np.asarray(W_dt, np.float32)
    b_dt = np.asarray(b_dt, np.float32)

    B, K = x.shape
    H = y.shape[1]
    Bs = B // N_CORES

    in_maps = pack_host_inputs(
        x, y, z, dt, W_ih, b_ih, W_hh, b_hh, W_z, b_z, b_dt, N_CORES
    )
    nc = build_nc(
        K,
        H,
        Bs,
        panel=Bs,
        chunk=512,
        wdt00=float(W_dt[0, 0]),
        wdt10=float(W_dt[1, 0]),
    )
    import os

    trace = os.environ.get("LEM_TRACE", "0") == "1"
    tmpdir = os.environ.get("LEM_TMPDIR") or None
    res = run_bass_kernel_spmd(
        nc, in_maps, list(range(N_CORES)), trace=trace, tmpdir=tmpdir
    )
    global LAST_RESULTS
    LAST_RESULTS = res
    y_newT = np.concatenate([r["y_newT"] for r in res.results], axis=1)
    z_newT = np.concatenate([r["z_newT"] for r in res.results], axis=1)
    return (
        np.ascontiguousarray(y_newT.T, dtype=np.float32),
        np.ascontiguousarray(z_newT.T, dtype=np.float32),
    )


# revision 34
# speedup vs baseline: 1.0224x; 1.0224x over previous
"""LEM cell (ODE2) Bass kernel for Trainium2, 8-core data-parallel, fp8 GEMMs.

Math (per batch row b):
  ti = x @ W_ih.T + b_ih                  # [B, 4H]
  th = y @ W_hh.T + b_hh                  # [B, 3H]
  tdt = dt @ W_dt.T + b_dt                # [B, 2]
  ms_dt_bar = sig(tdt[:,0]) * sig(ti[:, :H]   + th[:, :H])
  ms_dt     = sig(tdt[:,1]) * sig(ti[:, H:2H] + th[:, H:2H])
  z_new = (1-ms_dt) * z + ms_dt * tanh(ti[:, 3H:] + th[:, 2H:3H])
  y_new = (1-ms_dt_bar) * y + ms_dt_bar * tanh(z_new @ W_z.T + b_z + ti[:, 2H:3H])
  returns (y_new, z_new)

Strategy: shard batch across 8 cores (2048 rows each). On-chip everything is
feature-major ([feature_tile=128 partitions, batch columns free]); the host
pre-transposes activations and pre-packs weights into stationary blocks.
All GEMMs run in fp8 e4m3 with MatmulPerfMode.DoubleRow (two 128-row
contraction blocks per matmul, 2x bf16 PE rate). Activations are scaled
x16, weights x1024 before the e4m3 cast; the 2^-14 dequant rides in the
PSUM-consuming activation's `scale` operand. The i+h sums and the
i_z + z_new@W_z.T sum accumulate into the same PSUM bank. z / y for the
element-wise paths travel separately as fp32 / bf16, outputs fp32.

DMA descriptor generation costs ~620ns of sequencer time per dma_start,
so transfers are merged aggressively: x/y are host-packed pair-major
([pair, 128, 2, Bs]) so one DMA fills a DoubleRow rhs tile, weights load
as single full stationary tiles, z / y-elementwise load per-jt rows, and
outputs accumulate in half-panel SBUF tiles before one store each.
Measured fp8 error vs the fp32 reference: rel ~1.4e-2 (absmax metric).
"""

import sys

_REPO = "/opt/trn_rl_repo"
if _REPO not in sys.path:
    sys.path.insert(0, _REPO)

from contextlib import ExitStack

import numpy as np
import ml_dtypes

import concourse.bacc as bacc
import concourse.bass as bass
import concourse.tile as tile
from concourse import mybir
from concourse.bass_utils import run_bass_kernel_spmd

P = 128
F32 = mybir.dt.float32
F8 = mybir.dt.float8e4
BF16 = mybir.dt.bfloat16
AF = mybir.ActivationFunctionType
DR = mybir.MatmulPerfMode.DoubleRow
NP_F8 = ml_dtypes.float8_e4m3
NP_BF16 = ml_dtypes.bfloat16

N_CORES = 8
NINP = 1024
NHID = 1024
BATCH = 16384

SA = 16.0     # activation quant scale (|x|max ~5.6 -> 90 < 240)
SW = 1024.0   # weight quant scale (|w|max 1/32 -> 32 < 240)
DQ = 1.0 / (SA * SW)  # 2^-14, folded into the PSUM-reading activation

LAST_RESULTS = None  # BassKernelResults of the most recent kernel() call


def build_nc(
    K,            # input feature dim (x)
    H,            # hidden dim (y/z)
    B_shard,      # batch rows per core
    panel,        # batch columns kept resident per pass (== B_shard here)
    chunk,        # matmul moving-dim size (512 = one fp32 PSUM bank)
    wdt00, wdt10,  # W_dt scalars (baked immediates; b_dt rides in biasP)
    w_bufs=23,
    ps_bufs=8,
):
    NJT = H // P          # output feature tiles (per H-sized group)
    NKP = K // (2 * P)    # x-side contraction pair-tiles (DoubleRow)
    NHP = H // (2 * P)    # y/z-side contraction pair-tiles
    NWT = (K + H) // P    # weight tile columns (packed pairs layout)
    npan = B_shard // panel
    nch = panel // chunk
    half = panel // 2     # output tiles cover half panels

    nc = bacc.Bacc(trn_type="TRN2", target_bir_lowering=False)

    # pair-major activations: [pair, 128, 2, B] so one DMA fills a rhs tile
    x8P = nc.declare_dram_parameter("x8P", [NKP, P, 2, B_shard], F8, isOutput=False)
    y8P = nc.declare_dram_parameter("y8P", [NHP, P, 2, B_shard], F8, isOutput=False)
    ybT = nc.declare_dram_parameter("ybT", [H, B_shard], BF16, isOutput=False)
    zT = nc.declare_dram_parameter("zT", [H, B_shard], F32, isOutput=False)
    dtr = nc.declare_dram_parameter("dtr", [1, B_shard], F32, isOutput=False)
    # packed stationary blocks: [jt, kin, (kt_a*P+j | kt_b*P+j)] fp8
    Wd2 = nc.declare_dram_parameter("Wd2", [NJT, P, K + H], F8, isOutput=False)
    Wy = nc.declare_dram_parameter("Wy", [NJT, P, K + H], F8, isOutput=False)
    Wd1 = nc.declare_dram_parameter("Wd1", [NJT, P, K + H], F8, isOutput=False)
    Wg3 = nc.declare_dram_parameter("Wg3", [NJT, P, K + H], F8, isOutput=False)
    # last two columns: row 0 holds b_dt[0], b_dt[1]
    biasP = nc.declare_dram_parameter("biasP", [P, 4 * NJT + 2], F32, isOutput=False)

    y_newT = nc.declare_dram_parameter("y_newT", [H, B_shard], F32, isOutput=True)
    z_newT = nc.declare_dram_parameter("z_newT", [H, B_shard], F32, isOutput=True)

    assert npan == 1, "single-panel schedule (whole shard resident)"

    with tile.TileContext(nc) as tc, ExitStack() as ctx:
        cpool = ctx.enter_context(tc.tile_pool(name="cpool", bufs=1))
        xpool = ctx.enter_context(tc.tile_pool(name="xpool", bufs=NKP))
        ypool = ctx.enter_context(tc.tile_pool(name="ypool", bufs=NHP))
        ybpool = ctx.enter_context(tc.tile_pool(name="ybpool", bufs=2))
        zpool = ctx.enter_context(tc.tile_pool(name="zpool", bufs=2))
        znpool = ctx.enter_context(tc.tile_pool(name="znpool", bufs=NHP))
        wpool = ctx.enter_context(tc.tile_pool(name="wpool", bufs=w_bufs))
        apool = ctx.enter_context(tc.tile_pool(name="apool", bufs=3))
        dpool = ctx.enter_context(tc.tile_pool(name="dpool", bufs=3))
        opool = ctx.enter_context(tc.tile_pool(name="opool", bufs=2))
        bcpool = ctx.enter_context(tc.tile_pool(name="bcpool", bufs=1))
        rpool = ctx.enter_context(tc.tile_pool(name="rpool", bufs=2))
        pspool = ctx.enter_context(tc.tile_pool(name="pspool", bufs=ps_bufs, space="PSUM"))

        bias_sb = cpool.tile([P, 4 * NJT + 2], F32, name="bias_sb")
        nc.sync.dma_start(bias_sb[:], biasP[:, :])

        def bias_ap(g, jt):
            i = g * NJT + jt
            return bias_sb[:, i : i + 1]

        b0 = 0

        def col(c, n=1):
            return slice(b0 + c * chunk, b0 + (c + n) * chunk)

        dt_sb = rpool.tile([1, panel], F32, name="dt_sb", tag="dtr", bufs=1)
        nc.sync.dma_start(dt_sb[:], dtr[0:1, b0 : b0 + panel])

        # per-batch dt gates first: tiny ACT ops must precede the input
        # DMA flood in the ACT FIFO, else bc gates arrive ~40us late
        sg1 = rpool.tile([1, panel], F32, name="sg1", tag="sg")
        nc.scalar.activation(
            sg1[:], dt_sb[:], AF.Sigmoid,
            bias=bias_sb[0:1, 4 * NJT : 4 * NJT + 1], scale=wdt00,
        )
        sg2 = rpool.tile([1, panel], F32, name="sg2", tag="sg")
        nc.scalar.activation(
            sg2[:], dt_sb[:], AF.Sigmoid,
            bias=bias_sb[0:1, 4 * NJT + 1 : 4 * NJT + 2], scale=wdt10,
        )

        x_t = [None] * NKP
        y_t = [None] * NHP
        z_t = [None] * NJT
        yb_t = [None] * NJT

        def load_z(jt, eng):
            z_sb = zpool.tile([P, panel], F32, name="z_sb", tag="z")
            eng.dma_start(z_sb[:], zT[jt * P : (jt + 1) * P, b0 : b0 + panel])
            z_t[jt] = z_sb

        def load_yb(jt, eng):
            yb_sb = ybpool.tile([P, panel], BF16, name="yb_sb", tag="yb")
            eng.dma_start(yb_sb[:], ybT[jt * P : (jt + 1) * P, b0 : b0 + panel])
            yb_t[jt] = yb_sb

        def load_w(Wsrc, jt, name, eng):
            w_sb = wpool.tile([P, NWT, P], F8, name=name, tag="w")
            eng.dma_start(w_sb[:], Wsrc[jt][:, :])
            return w_sb

        # ---- cold-start staging ----
        # x pair-tiles span the whole panel; q0's first chunk arrives first
        # (gpsimd), everything else streams on the otherwise-idle DVE queue
        for q in range(NKP):
            x_t[q] = xpool.tile([P, 2, panel], F8, name="xt", tag="xt")
        # x pair q0 leads on gpsimd; q1-3 spread over sync/scalar so their
        # transfers run in parallel with q0's (per-queue DMA is serial)
        nc.gpsimd.dma_start(x_t[0][:, :, 0:chunk], x8P[0][:, :, 0:chunk])
        nc.gpsimd.dma_start(x_t[0][:, :, chunk:panel], x8P[0][:, :, chunk:panel])

        # spread the cold-start bytes across all three DMA queues in
        # consumption order (per-queue transfers drain serially ~200GB/s)
        for q in range(NHP):
            y_t[q] = ypool.tile([P, 2, panel], F8, name="yt", tag="yt")
        wB = [[None, None] for _ in range(NJT)]   # per jt: [Wd2, Wy]
        wC = [[None, None] for _ in range(NJT)]   # per jt: [Wd1, Wg3]
        wB[0][0] = load_w(Wd2, 0, "wd2_sb", nc.sync)
        wB[0][1] = load_w(Wy, 0, "wy_sb", nc.scalar)
        for q in range(1, NKP):
            nc.scalar.dma_start(x_t[q][:], x8P[q][:, :, :])
        for q in range(NHP):
            nc.sync.dma_start(y_t[q][:], y8P[q][:, :, :])
        # bc broadcasts ahead of the bulky z loads (bc2 is needed by the
        # first DVE op at ~24us, z0 not till ~26us)
        bc1 = bcpool.tile([P, panel], F32, name="bc1", tag="bc1")
        nc.gpsimd.partition_broadcast(bc1[:], sg1[0:1, :])
        bc2 = bcpool.tile([P, panel], F32, name="bc2", tag="bc2")
        nc.gpsimd.partition_broadcast(bc2[:], sg2[0:1, :])
        load_z(0, nc.gpsimd)
        load_z(1, nc.gpsimd)
        for jt in range(1, NJT):
            wB[jt][0] = load_w(Wd2, jt, "wd2_sb", nc.sync)
            wB[jt][1] = load_w(Wy, jt, "wy_sb", nc.scalar)

        kb = K // P

        def x_of(q, c):
            return x_t[q][:, :, c * chunk : (c + 1) * chunk]

        def y_of(q, c):
            return y_t[q][:, :, c * chunk : (c + 1) * chunk]

        def zn_of(q, c):
            return zn_t[q][:, :, c * chunk : (c + 1) * chunk]

        def accum_blocks(pss, w_sb, rhs_of, side, n_q, start=False, stop=False):
            """Block-outer fp8 DoubleRow accumulation: for each stationary
            pair-block q, matmul all chunks' psums before moving on, so the
            PE array reuses the loaded weights across `nch` moving passes."""
            base = 0 if side == 0 else kb
            for q in range(n_q):
                lhsT = w_sb[:, base + 2 * q : base + 2 * q + 2, :]
                for c in range(len(pss)):
                    nc.tensor.matmul(
                        pss[c][:],
                        lhsT=lhsT,
                        rhs=rhs_of(q, c),
                        start=start and (q == 0),
                        stop=stop and (q == n_q - 1),
                        perf_mode=DR,
                    )

        # ---- phase B: d2 + y gates -> z_new ----
        # zn pair-tiles [P, 2, panel]: pair q holds jt=2q / 2q+1 rows
        zn_t = [
            znpool.tile([P, 2, panel], F8, name="znr", tag="zn")
            for _ in range(NHP)
        ]
        for jt in range(NJT):
            wd2_sb, wy_sb = wB[jt]
            if jt + 2 < NJT:
                load_z(jt + 2, nc.gpsimd)
            elif jt + 2 - NJT < NJT:
                load_yb(jt + 2 - NJT, nc.gpsimd)  # yb0/yb1 prefetch late in B
            # stage phase-C stationary tiles through B's back half
            if 2 <= jt < 2 + NJT // 2:
                j2 = 2 * (jt - 2)
                wC[j2][0] = load_w(Wd1, j2, "wd1_sb", nc.sync)
                wC[j2][1] = load_w(Wg3, j2, "wg3_sb", nc.scalar)
                wC[j2 + 1][0] = load_w(Wd1, j2 + 1, "wd1_sb", nc.sync)
                wC[j2 + 1][1] = load_w(Wg3, j2 + 1, "wg3_sb", nc.scalar)
            zo = opool.tile([P, panel], F32, name="zo", tag="zo")
            ps1s = [pspool.tile([P, chunk], F32, name="ps1", tag="ps") for _ in range(nch)]
            ps2s = [pspool.tile([P, chunk], F32, name="ps2", tag="ps") for _ in range(nch)]
            # x-sides first (block-outer): PE chews these while y streams
            accum_blocks(ps1s, wd2_sb, x_of, 0, NKP, start=True)
            accum_blocks(ps2s, wy_sb, x_of, 0, NKP, start=True)
            accum_blocks(ps1s, wd2_sb, y_of, 1, NHP, stop=True)
            accum_blocks(ps2s, wy_sb, y_of, 1, NHP, stop=True)
            for c in range(nch):
                cs = slice(c * chunk, (c + 1) * chunk)
                s2 = apool.tile([P, chunk], F32, name="s2", tag="act")
                nc.scalar.activation(s2[:], ps1s[c][:], AF.Sigmoid, bias=bias_ap(0, jt), scale=DQ)
                tz = apool.tile([P, chunk], F32, name="tz", tag="act")
                nc.scalar.activation(tz[:], ps2s[c][:], AF.Tanh, bias=bias_ap(1, jt), scale=DQ)

                # z_new = z + ms2*(tanh - z); 4 DVE ops
                ms2 = dpool.tile([P, chunk], F32, name="ms2", tag="dve")
                nc.vector.tensor_mul(ms2[:], s2[:], bc2[:, cs])
                dlt = dpool.tile([P, chunk], F32, name="dlt", tag="dve")
                nc.vector.tensor_sub(dlt[:], tz[:], z_t[jt][:, cs])
                prd = dpool.tile([P, chunk], F32, name="prd", tag="dve")
                nc.vector.tensor_mul(prd[:], ms2[:], dlt[:])
                nc.vector.tensor_add(zo[:, cs], prd[:], z_t[jt][:, cs])
                # quantizing cast (x16 -> e4m3) into the resident
                # DoubleRow pair-tile for GEMM3
                nc.scalar.mul(zn_t[jt // 2][:, jt % 2, cs], zo[:, cs], SA)
                if c % 2 == 1:
                    nc.sync.dma_start(
                        z_newT[jt * P : (jt + 1) * P, col(c - 1, 2)],
                        zo[:, (c - 1) * chunk : (c + 1) * chunk],
                    )

        # ---- phase C: d1 gate + (i_z + z_new @ W_z.T) -> y_new ----
        for jt in range(NJT):
            if wC[jt][0] is None:
                wC[jt][0] = load_w(Wd1, jt, "wd1_sb", nc.sync)
                wC[jt][1] = load_w(Wg3, jt, "wg3_sb", nc.scalar)
            wd1_sb, wg3_sb = wC[jt]
            if jt + 2 < NJT:
                load_yb(jt + 2, nc.gpsimd)
            yo = opool.tile([P, panel], F32, name="yo", tag="yo")

            def c_tail(c, ps3, ps4):
                cs = slice(c * chunk, (c + 1) * chunk)
                s1 = apool.tile([P, chunk], F32, name="s1", tag="act")
                nc.scalar.activation(s1[:], ps3[:], AF.Sigmoid, bias=bias_ap(2, jt), scale=DQ)
                u = apool.tile([P, chunk], F32, name="u", tag="act")
                nc.scalar.activation(u[:], ps4[:], AF.Tanh, bias=bias_ap(3, jt), scale=DQ)

                # y_new = y + ms1*(u - y); 4 DVE ops
                ms1 = dpool.tile([P, chunk], F32, name="ms1", tag="dve")
                nc.vector.tensor_mul(ms1[:], s1[:], bc1[:, cs])
                dly = dpool.tile([P, chunk], F32, name="dly", tag="dve")
                nc.vector.tensor_sub(dly[:], u[:], yb_t[jt][:, cs])
                mdy = dpool.tile([P, chunk], F32, name="mdy", tag="dve")
                nc.vector.tensor_mul(mdy[:], ms1[:], dly[:])
                nc.vector.tensor_add(yo[:, cs], mdy[:], yb_t[jt][:, cs])
                if c % 2 == 1:
                    nc.scalar.dma_start(
                        y_newT[jt * P : (jt + 1) * P, col(c - 1, 2)],
                        yo[:, (c - 1) * chunk : (c + 1) * chunk],
                    )

            if jt < NJT - 1:
                ps3s = [pspool.tile([P, chunk], F32, name="ps3", tag="ps") for _ in range(nch)]
                ps4s = [pspool.tile([P, chunk], F32, name="ps4", tag="ps") for _ in range(nch)]
                accum_blocks(ps3s, wd1_sb, x_of, 0, NKP, start=True)
                accum_blocks(ps4s, wg3_sb, x_of, 0, NKP, start=True)
                accum_blocks(ps3s, wd1_sb, y_of, 1, NHP, stop=True)
                accum_blocks(ps4s, wg3_sb, zn_of, 1, NHP, stop=True)
                for c in range(nch):
                    c_tail(c, ps3s[c], ps4s[c])
            else:
                # last jt: chunk-interleaved with per-chunk stores so the
                # ACT/DVE/store tail overlaps the remaining matmuls
                for c in range(nch):
                    cs = slice(c * chunk, (c + 1) * chunk)
                    ps3 = pspool.tile([P, chunk], F32, name="ps3", tag="ps")
                    accum_blocks([ps3], wd1_sb, lambda q, _c, c=c: x_of(q, c), 0, NKP, start=True)
                    accum_blocks([ps3], wd1_sb, lambda q, _c, c=c: y_of(q, c), 1, NHP, stop=True)
                    ps4 = pspool.tile([P, chunk], F32, name="ps4", tag="ps")
                    accum_blocks([ps4], wg3_sb, lambda q, _c, c=c: x_of(q, c), 0, NKP, start=True)
                    accum_blocks([ps4], wg3_sb, lambda q, _c, c=c: zn_of(q, c), 1, NHP, stop=True)
                    s1 = apool.tile([P, chunk], F32, name="s1", tag="act")
                    nc.scalar.activation(s1[:], ps3[:], AF.Sigmoid, bias=bias_ap(2, jt), scale=DQ)
                    u = apool.tile([P, chunk], F32, name="u", tag="act")
                    nc.scalar.activation(u[:], ps4[:], AF.Tanh, bias=bias_ap(3, jt), scale=DQ)
                    ve = nc.vector
                    ms1 = dpool.tile([P, chunk], F32, name="ms1", tag="dve")
                    ve.tensor_mul(ms1[:], s1[:], bc1[:, cs])
                    if c < nch - 1:
                        dly = dpool.tile([P, chunk], F32, name="dly", tag="dve")
                        ve.tensor_sub(dly[:], u[:], yb_t[jt][:, cs])
                        mdy = dpool.tile([P, chunk], F32, name="mdy", tag="dve")
                        ve.tensor_mul(mdy[:], ms1[:], dly[:])
                        ve.tensor_add(yo[:, cs], mdy[:], yb_t[jt][:, cs])
                        nc.scalar.dma_start(
                            y_newT[jt * P : (jt + 1) * P, col(c)], yo[:, cs]
                        )
                    else:
                        # final chunk: (1-ms1)*y precomputed before the tanh
                        # lands, then a fused finish with one half on each
                        # engine so the last bytes store as early as possible
                        my = dpool.tile([P, chunk], F32, name="my", tag="dve")
                        nc.vector.tensor_mul(my[:], ms1[:], yb_t[jt][:, cs])
                        wyp = dpool.tile([P, chunk], F32, name="wyp", tag="dve")
                        nc.vector.tensor_sub(wyp[:], yb_t[jt][:, cs], my[:])
                        hf = chunk // 2
                        for h in range(2):
                            he = nc.vector
                            so = nc.sync if h == 0 else nc.scalar
                            hs = slice(c * chunk + h * hf, c * chunk + (h + 1) * hf)
                            ho = slice(h * hf, (h + 1) * hf)
                            mu = dpool.tile([P, hf], F32, name="mu", tag="dvef", bufs=2)
                            he.tensor_mul(mu[:], ms1[:, ho], u[:, ho])
                            he.tensor_add(yo[:, hs], wyp[:, ho], mu[:])
                            so.dma_start(
                                y_newT[jt * P : (jt + 1) * P,
                                       b0 + c * chunk + h * hf : b0 + c * chunk + (h + 1) * hf],
                                yo[:, hs],
                            )

    nc.compile()
    return nc


def _q8(a, s):
    """Scale and round-to-nearest cast to e4m3."""
    return (np.asarray(a, np.float32) * s).astype(NP_F8)


def _pack_pair(Wa, Wb):
    """[jt, kin, kt*P+j] stationary-block packing of two row-major [out, in]
    weight matrices (lhsT blocks: lhsT[kin, j] = W[jt*P+j, kt*P+kin]).
    Consecutive kt pairs are exactly the DoubleRow [kin, 2, j] layout."""
    def pack(W):
        O, I = W.shape
        njt, nkt = O // P, I // P
        return (
            W.reshape(njt, P, nkt, P).transpose(0, 3, 2, 1).reshape(njt, P, I)
        )
    A = pack(Wa)
    B = pack(Wb)
    return np.ascontiguousarray(np.concatenate([A, B], axis=2))


def _pair_major(a8T):
    """[K, B] fp8 -> [K/256, 128, 2, B] DoubleRow pair-major layout."""
    K, B = a8T.shape
    return np.ascontiguousarray(
        a8T.reshape(K // 256, 2, P, B).transpose(0, 2, 1, 3)
    )


def pack_host_inputs(x, y, z, dt, W_ih, b_ih, W_hh, b_hh, W_z, b_z, b_dt, n_cores):
    """Shard batch across cores; pre-transpose + fp8-quantize activations;
    pack + fp8-quantize weights."""
    B, K = x.shape
    H = y.shape[1]
    NJT = H // P
    Bs = B // n_cores

    x8T = np.ascontiguousarray(_q8(x, SA).T)
    y8T = np.ascontiguousarray(_q8(y, SA).T)
    ybT = np.ascontiguousarray(np.asarray(y, np.float32).T.astype(NP_BF16))
    zT = np.ascontiguousarray(np.asarray(z, np.float32).T)
    dtrow = np.ascontiguousarray(np.asarray(dt, np.float32).reshape(1, B))

    Wd2 = _pack_pair(_q8(W_ih[H : 2 * H], SW), _q8(W_hh[H : 2 * H], SW))
    Wy = _pack_pair(_q8(W_ih[3 * H : 4 * H], SW), _q8(W_hh[2 * H : 3 * H], SW))
    Wd1 = _pack_pair(_q8(W_ih[0:H], SW), _q8(W_hh[0:H], SW))
    Wg3 = _pack_pair(_q8(W_ih[2 * H : 3 * H], SW), _q8(W_z, SW))

    def bias_cols(bvec):
        return bvec.reshape(NJT, P).T  # [P, NJT]

    bdt_cols = np.zeros((P, 2), np.float32)
    bdt_cols[0, 0] = b_dt[0]
    bdt_cols[0, 1] = b_dt[1]
    biasP = np.ascontiguousarray(
        np.concatenate(
            [
                bias_cols(b_ih[H : 2 * H] + b_hh[H : 2 * H]),
                bias_cols(b_ih[3 * H : 4 * H] + b_hh[2 * H : 3 * H]),
                bias_cols(b_ih[0:H] + b_hh[0:H]),
                bias_cols(b_ih[2 * H : 3 * H] + b_z),
                bdt_cols,
            ],
            axis=1,
        ),
        dtype=np.float32,
    )

    in_maps = []
    for c in range(n_cores):
        cs = slice(c * Bs, (c + 1) * Bs)
        in_maps.append(
            {
                "x8P": _pair_major(x8T[:, cs]),
                "y8P": _pair_major(y8T[:, cs]),
                "ybT": np.ascontiguousarray(ybT[:, cs]),
                "zT": np.ascontiguousarray(zT[:, cs]),
                "dtr": np.ascontiguousarray(dtrow[:, cs]),
                "Wd2": Wd2,
                "Wy": Wy,
                "Wd1": Wd1,
                "Wg3": Wg3,
                "biasP": biasP,
            }
        )
    return in_maps


def kernel(x, y, z, dt, W_ih, b_ih, W_hh, b_hh, W_z, b_z, W_dt, b_dt):
    x = np.asarray(x, np.float32)
    y = np.asarray(y, np.float32)
    z = np.asarray(z, np.float32)
    dt = np.asarray(dt, np.float32)
    W_ih = np.asarray(W_ih, np.float32)
    b_ih = np.asarray(b_ih, np.float32)
    W_hh = np.asarray(W_hh, np.float32)
    b_hh = np.asarray(b_hh, np.float32)
    W_z = np.asarray(W_z, np.float32)
    b_z = np.asarray(b_z, np.float32)
    W_dt = np.asarray(W_dt, np.float32)
    b_dt = np.asarray(b_dt, np.float32)

    B, K = x.shape
    H = y.shape[1]
    Bs = B // N_CORES

    in_maps = pack_host_inputs(
        x, y, z, dt, W_ih, b_ih, W_hh, b_hh, W_z, b_z, b_dt, N_CORES
    )
    nc = build_nc(
        K,
        H,
        Bs,
        panel=Bs,
        chunk=512,
        wdt00=float(W_dt[0, 0]),
        wdt10=float(W_dt[1, 0]),
    )
    import os

    trace = os.environ.get("LEM_TRACE", "0") == "1"
    tmpdir = os.environ.get("LEM_TMPDIR") or None
    res = run_bass_kernel_spmd(
        nc, in_maps, list(range(N_CORES)), trace=trace, tmpdir=tmpdir
    )
    global LAST_RESULTS
    LAST_RESULTS = res
    y_newT = np.concatenate([r["y_newT"] for r in res.results], axis=1)
    z_newT = np.concatenate([r["z_newT"] for r in res.results], axis=1)
    return (
        np.ascontiguousarray(y_newT.T, dtype=np.float32),
        np.ascontiguousarray(z_newT.T, dtype=np.float32),
    )
